# revision 1
# baseline (speedup 1.0000x reference)
"""Dilated attention TRN2 kernel (nn_DilatedAttention_full).

Full inputs q/k/v: [B*H=32, L=2048, D=64] f32. Output [B=4, L=2048, H*D=512].

Sharding: 32 (b,h) pairs -> 8 cores x 4 pairs. Host pre-gathers each
dilation branch (dr in [1,2,4,8]; head h uses rows r_h(dr) = h//(8//dr) mod
dr) so each branch is a dense, unscaled-softmax full attention of length
L/dr. The kernel computes, per pair and branch, O = softmax(Q K^T) V
(normalization applied in-kernel via a ones-column in V that yields the
softmax denominators for free). Host scatters branch outputs back to their
dilated row positions and sums overlapping branches.
"""
import sys
sys.path.insert(0, '/opt/trn_rl_repo')
import numpy as np

import concourse.bass as bass
from concourse import bacc
import concourse.tile as tile
from concourse import mybir
from concourse.bass_utils import run_bass_kernel_spmd
from concourse.masks import make_identity

F32 = mybir.dt.float32
F32R = mybir.dt.float32r
EXP = mybir.ActivationFunctionType.Exp
MULT = mybir.AluOpType.mult

B, H, L, D = 4, 8, 2048, 64
N_CORES = 8
PAIRS = 4              # (b,h) pairs per core
DRS = [1, 2, 4, 8]
LSS = [L // dr for dr in DRS]           # 2048 1024 512 256
OFFS = [0, 2048, 3072, 3584]            # branch row offsets in packed input
TOT = sum(LSS)                          # 3840


def _build_kernel_body(tc, q_ap, k_ap, v_ap, o_ap):
    nc = tc.nc
    ctx_pools = []

    def pool(name, bufs, space="SBUF"):
        p = tc.tile_pool(name=name, bufs=bufs, space=space)
        ctx_pools.append(p)
        return p.__enter__()

    cpool = pool("const", 1)
    qkt_pool = pool("qkt", 2)
    vp_pool = pool("vp", 2)
    stage_pool = pool("stage", 2)
    p_pool = pool("pmat", 3)
    ot_pool = pool("osb", 2)
    rec_pool = pool("rec", 2)
    out_pool = pool("ostage", 2)
    s_pool = pool("s", 2, "PSUM")
    o_pool = pool("oacc", 1, "PSUM")
    t_pool = pool("tpsum", 1, "PSUM")

    ident = cpool.tile([128, 128], F32)
    make_identity(nc, ident[:])

    for pp in range(PAIRS // 2):
        pa, pb = 2 * pp, 2 * pp + 1
        for di, dr in enumerate(DRS):
            ls = LSS[di]
            nt = ls // 128
            off = OFFS[di]

            # ---------- prep: Q^T/K^T (pair A on parts 0:64, B on 64:128) ----------
            qt = qkt_pool.tile([128, ls], F32, tag="qt")
            kt = qkt_pool.tile([128, ls], F32, tag="kt")
            for role_ap, dst in ((q_ap, qt), (k_ap, kt)):
                stage = stage_pool.tile([128, nt * 128], F32, tag="qkstage")
                s3 = stage[:].rearrange("p (t c) -> p t c", c=128)
                nc.sync.dma_start(
                    s3[:, :, 0:64],
                    role_ap[pa, off:off + ls, :].rearrange("(t p) d -> p t d", p=128))
                nc.sync.dma_start(
                    s3[:, :, 64:128],
                    role_ap[pb, off:off + ls, :].rearrange("(t p) d -> p t d", p=128))
                for t0 in range(0, nt, 4):
                    g = min(4, nt - t0)
                    tp = t_pool.tile([128, 512], F32, tag="tp")
                    for j in range(g):
                        nc.tensor.transpose(
                            tp[:, j * 128:(j + 1) * 128],
                            stage[:, (t0 + j) * 128:(t0 + j + 1) * 128],
                            ident[:])
                    nc.vector.tensor_copy(
                        dst[:, t0 * 128:(t0 + g) * 128].bitcast(F32R),
                        tp[:, 0:g * 128])

            # ---------- prep: V' = [V | ones] per pair, [128, nt, 65] ----------
            vps = []
            for slot in (pa, pb):
                vstage = stage_pool.tile([128, nt * 65], F32, tag="vstage")
                nc.gpsimd.memset(vstage[:], 1.0)
                nc.sync.dma_start(
                    vstage[:].rearrange("p (t e) -> p t e", e=65)[:, :, 0:64],
                    v_ap[slot, off:off + ls, :].rearrange("(t p) d -> p t d", p=128))
                vp = vp_pool.tile([128, nt * 65], F32, tag=("va" if slot == pa else "vb"))
                nc.vector.tensor_copy(vp[:].bitcast(F32R), vstage[:])
                vps.append(vp[:].rearrange("p (t e) -> p t e", e=65))
            vpa, vpb = vps

            # ---------- compute ----------
            cw = min(512, ls)
            n_chunks = ls // cw
            for ci in range(n_chunks):
                c0 = ci * cw
                oa = o_pool.tile([65, cw], F32, tag="oa")
                ob = o_pool.tile([65, cw], F32, tag="ob")
                for kti in range(nt):
                    s = s_pool.tile([128, 1024], F32, tag="s")
                    nc.tensor.matmul(
                        s[:, 0:cw],
                        kt[0:64, kti * 128:(kti + 1) * 128].bitcast(F32R),
                        qt[0:64, c0:c0 + cw].bitcast(F32R),
                        start=True, stop=True, tile_position=(0, 0))
                    nc.tensor.matmul(
                        s[:, 512:512 + cw],
                        kt[64:128, kti * 128:(kti + 1) * 128].bitcast(F32R),
                        qt[64:128, c0:c0 + cw].bitcast(F32R),
                        start=True, stop=True, tile_position=(64, 0))
                    p = p_pool.tile([128, 1024], F32, tag="p")
                    s_in = s[:].rearrange("x (two c) -> x two c", two=2)[:, :, 0:cw]
                    p_out = p[:, 0:2 * cw].rearrange("x (two c) -> x two c", two=2)
                    nc.scalar.activation(p_out.bitcast(F32R), s_in, EXP)
                    first, last = kti == 0, kti == nt - 1
                    nc.tensor.matmul(
                        oa[:], vpa[:, kti, :].bitcast(F32R),
                        p[:, 0:cw].bitcast(F32R), start=first, stop=last)
                    nc.tensor.matmul(
                        ob[:], vpb[:, kti, :].bitcast(F32R),
                        p[:, cw:2 * cw].bitcast(F32R), start=first, stop=last)

                # ---------- normalize + transpose back + store ----------
                nu = cw // 128
                for slot, oacc in ((pa, oa), (pb, ob)):
                    osb = ot_pool.tile([65, cw], F32, tag="osb")
                    nc.vector.tensor_copy(osb[:], oacc[:])
                    otp = t_pool.tile([128, 260], F32, tag="otp")
                    for u in range(nu):
                        nc.tensor.transpose(
                            otp[:, u * 65:(u + 1) * 65],
                            osb[:, u * 128:(u + 1) * 128],
                            ident[0:65, 0:65])
                    rec = rec_pool.tile([128, 4], F32, tag="rec")
                    nc.vector.reciprocal(
                        rec[:, 0:nu],
                        otp[:].rearrange("p (u e) -> p u e", e=65)[:, 0:nu, 64])
                    ostage = out_pool.tile([128, 256], F32, tag="ostage")
                    for u in range(nu):
                        nc.vector.tensor_scalar(
                            ostage[:, u * 64:(u + 1) * 64],
                            otp[:, u * 65:u * 65 + 64],
                            rec[:, u:u + 1], None, op0=MULT)
                    nc.sync.dma_start(
                        o_ap[slot, off + c0:off + c0 + cw, :].rearrange(
                            "(u p) d -> p u d", p=128),
                        ostage[:, 0:nu * 64].rearrange("p (u d) -> p u d", d=64))

    for p in reversed(ctx_pools):
        p.__exit__(None, None, None)


_NC_CACHE = None


def _build_module():
    global _NC_CACHE
    if _NC_CACHE is not None:
        return _NC_CACHE
    nc = bacc.Bacc("TRN2", target_bir_lowering=False, debug=False)
    q_ap = nc.dram_tensor("q", [PAIRS, TOT, D], F32, kind="ExternalInput").ap()
    k_ap = nc.dram_tensor("k", [PAIRS, TOT, D], F32, kind="ExternalInput").ap()
    v_ap = nc.dram_tensor("v", [PAIRS, TOT, D], F32, kind="ExternalInput").ap()
    o_ap = nc.dram_tensor("o", [PAIRS, TOT, D], F32, kind="ExternalOutput").ap()
    with tile.TileContext(nc) as tc:
        _build_kernel_body(tc, q_ap, k_ap, v_ap, o_ap)
    nc.compile()
    _NC_CACHE = nc
    return nc


def _gather_branches(x_bh):
    """x_bh: [L, D] for one (b,h); returns [TOT, D] packed branches."""
    return x_bh  # placeholder, not used


def _pack_inputs(query, key, value):
    """Returns in_maps: list of N_CORES dicts with q/k/v [PAIRS, TOT, D]."""
    in_maps = []
    for c in range(N_CORES):
        qm = np.empty((PAIRS, TOT, D), np.float32)
        km = np.empty((PAIRS, TOT, D), np.float32)
        vm = np.empty((PAIRS, TOT, D), np.float32)
        for i in range(PAIRS):
            bh = 4 * c + i
            h = bh % H
            for di, dr in enumerate(DRS):
                r = h // (H // dr)
                sl = slice(OFFS[di], OFFS[di] + LSS[di])
                qm[i, sl] = query[bh, r::dr]
                km[i, sl] = key[bh, r::dr]
                vm[i, sl] = value[bh, r::dr]
        in_maps.append({"q": qm, "k": km, "v": vm})
    return in_maps


def _unpack_outputs(results):
    """results: list of N_CORES dicts with 'o' [PAIRS, TOT, D] ->
    full output [B, L, H*D]."""
    out = np.zeros((B, L, H, D), np.float32)
    for c in range(N_CORES):
        o = results[c]["o"]
        for i in range(PAIRS):
            bh = 4 * c + i
            b, h = bh // H, bh % H
            for di, dr in enumerate(DRS):
                r = h // (H // dr)
                sl = slice(OFFS[di], OFFS[di] + LSS[di])
                out[b, r::dr, h] += o[i, sl]
    return out.reshape(B, L, H * D)


def kernel(query, key, value):
    query = np.asarray(query, dtype=np.float32)
    key = np.asarray(key, dtype=np.float32)
    value = np.asarray(value, dtype=np.float32)
    nc = _build_module()
    in_maps = _pack_inputs(query, key, value)
    res = run_bass_kernel_spmd(nc, in_maps, core_ids=list(range(N_CORES)))
    return _unpack_outputs(res.results)


# revision 21
# speedup vs baseline: 206.1515x; 206.1515x over previous
"""Dilated attention TRN2 kernel (nn_DilatedAttention_full).

Full inputs q/k/v: [B*H=32, L=2048, D=64] f32. Output [B=4, L=2048, H*D=512].

Sharding: 32 (b,h) pairs -> 8 cores x 4 pairs. Host pre-gathers each
dilation branch (dr in [1,2,4,8]; head h uses rows r_h(dr) = h//(8//dr) mod
dr) so each branch is a dense, unscaled-softmax full attention of length
L/dr. The kernel computes, per pair and branch, O = softmax(Q K^T) V
(normalization applied in-kernel via a ones-column in V that yields the
softmax denominators for free). Host scatters branch outputs back to their
dilated row positions and sums overlapping branches.
"""
import sys
sys.path.insert(0, '/opt/trn_rl_repo')
import numpy as np

import concourse.bass as bass
from concourse import bacc
import concourse.tile as tile
from concourse import mybir
from concourse.bass_utils import run_bass_kernel_spmd
from concourse.masks import make_identity

F32 = mybir.dt.float32
F32R = mybir.dt.float32r
EXP = mybir.ActivationFunctionType.Exp
MULT = mybir.AluOpType.mult

B, H, L, D = 4, 8, 2048, 64
N_CORES = 8
PAIRS = 4              # (b,h) pairs per core
DRS = [1, 2, 4, 8]
LSS = [L // dr for dr in DRS]           # 2048 1024 512 256
OFFS = [0, 2048, 3072, 3584]            # branch row offsets in packed input
TOT = sum(LSS)                          # 3840


def _build_kernel_body(tc, q_ap, k_ap, v_ap, o_ap, gate_small=0.030, gate_v=0.008):
    nc = tc.nc
    ctx_pools = []

    def pool(name, bufs, space="SBUF"):
        p = tc.tile_pool(name=name, bufs=bufs, space=space)
        ctx_pools.append(p)
        return p.__enter__()

    cpool = pool("const", 1)
    qkt_pool = pool("qkt", 2)
    vp_pool = pool("vp", 2)
    stage_pool = pool("stage", 1)
    p_pool = pool("pmat", 3)
    ot_pool = pool("osb", 2)
    rec_pool = pool("rec", 2)
    out_pool = pool("ostage", 2)
    s_pool = pool("s", 2, "PSUM")
    o_pool = pool("oacc", 1, "PSUM")
    t_pool = pool("tpsum", 1, "PSUM")

    ident = cpool.tile([128, 128], F32)
    make_identity(nc, ident[:])

    # Deferred output emission: emit chunk c's output phase after chunk c+1's
    # first matmuls, so the scheduler keeps the QK->exp->PV pipeline hot.
    pending_out = []

    def flush_pending():
        for fn in pending_out:
            fn()
        pending_out.clear()

    def emit_output(pa, pb, oa, ob, off, c0, cw):
        nu = cw // 128
        for slot, oacc in ((pa, oa), (pb, ob)):
            osb = ot_pool.tile([65, cw], F32, tag="osb")
            nc.vector.tensor_copy(osb[:], oacc[:])
            otp = t_pool.tile([128, 260], F32, tag="otp")
            for u in range(nu):
                nc.tensor.transpose(
                    otp[:, u * 65:(u + 1) * 65],
                    osb[:, u * 128:(u + 1) * 128],
                    ident[0:65, 0:65])
            rec = rec_pool.tile([128, 4], F32, tag="rec")
            nc.vector.reciprocal(
                rec[:, 0:nu],
                otp[:].rearrange("p (u e) -> p u e", e=65)[:, 0:nu, 64])
            ostage = out_pool.tile([128, 256], F32, tag="ostage")
            for u in range(nu):
                nc.vector.tensor_scalar(
                    ostage[:, u * 64:(u + 1) * 64],
                    otp[:, u * 65:u * 65 + 64],
                    rec[:, u:u + 1], None, op0=MULT)
            # output DMAs go via SWDGE (Pool engine, otherwise idle) so they
            # never block the input loads on the SP ring or the ACT engine
            nc.gpsimd.dma_start(
                o_ap[slot, off + c0:off + c0 + cw, :].rearrange(
                    "(u p) d -> p u d", p=128),
                ostage[:, 0:nu * 64].rearrange("p (u d) -> p u d", d=64))

    # Sections: per pair-pair, the small branches (dr 2/4/8; rows 2048:3840 are
    # contiguous in the packed input) load with ONE DMA per (role, pair) into a
    # combined Q^T/K^T/V' tile; dr=1 gets its own. Small branches run first so
    # the ACT pipeline ramps while the big branch's prep streams in.
    NT_S = 14          # tiles in small section (8 + 4 + 2)
    SOFF = 2048        # input row offset of small section
    TILE_OFF = {1: 0, 2: 0, 4: 8, 8: 12}

    for pp in range(PAIRS // 2):
        pa, pb = 2 * pp, 2 * pp + 1

        # ---------- section loads (DMAs only) ----------
        # sec 0 = big (dr1, rows 0:2048, 16 tiles), sec 1 = small (rows 2048:3840)
        # The small section's loads are EMITTED during dr1's second chunk so
        # they don't contend with the startup-critical big loads / V loads.
        sec_tiles = {}

        def load_section(sec, row0, ntt, pieces):
            tg = f"s{sec}"
            qt = qkt_pool.tile([128, ntt * 128], F32, tag="qt" + tg)
            kt = qkt_pool.tile([128, ntt * 128], F32, tag="kt" + tg)
            qstage = stage_pool.tile([128, ntt * 128], F32, tag="qstage" + tg)
            kstage = stage_pool.tile([128, ntt * 128], F32, tag="kstage" + tg)
            for ta, tb in pieces:
                # K before Q: every l-chunk consumes all key tiles, so K
                # availability gates the pipeline hardest
                for role_ap, stage in ((k_ap, kstage), (q_ap, qstage)):
                    s3 = stage[:].rearrange("p (t c) -> p t c", c=128)
                    src_rows = role_ap[:, row0 + ta * 128:row0 + tb * 128, :]
                    nc.sync.dma_start(
                        s3[:, ta:tb, 0:64],
                        src_rows[pa].rearrange("(t p) d -> p t d", p=128))
                    nc.sync.dma_start(
                        s3[:, ta:tb, 64:128],
                        src_rows[pb].rearrange("(t p) d -> p t d", p=128))
            vstages = []
            for vsi, slot in enumerate((pa, pb)):
                vstage = stage_pool.tile([128, ntt * 65], F32, tag=f"vs{vsi}" + tg)
                nc.gpsimd.memset(
                    vstage[:].rearrange("p (t e) -> p t e", e=65)[:, :, 64:65], 1.0)
                for ta, tb in pieces:
                    nc.gpsimd.dma_start(
                        vstage[:].rearrange("p (t e) -> p t e", e=65)[:, ta:tb, 0:64],
                        v_ap[slot, row0 + ta * 128:row0 + tb * 128, :]
                        .rearrange("(t p) d -> p t d", p=128))
                vstages.append(vstage)
            vpa_t = vp_pool.tile([128, ntt * 65], F32, tag="va" + tg)
            vpb_t = vp_pool.tile([128, ntt * 65], F32, tag="vb" + tg)
            sec_tiles[sec] = (qt, kt, qstage, kstage, vstages, vpa_t, vpb_t, ntt)

        load_section(0, 0, 16, [(0, 4), (4, 8), (8, 16)])

        # lazy prep: transpose 4-tile groups / V'-copy halves at first use,
        # emitted inline so the PE stream never parks on an unloaded piece
        done_t = set()
        done_v = set()

        def ensure_t(sec, role, tile_idx):
            ntt = sec_tiles[sec][7]
            g0 = (tile_idx // 4) * 4
            g1 = min(g0 + 4, ntt)
            key = (sec, role, g0)
            if key in done_t:
                return
            done_t.add(key)
            qt, kt, qstage, kstage, _, _, _, _ = sec_tiles[sec]
            stage, dst = (qstage, qt) if role == 'q' else (kstage, kt)
            tp = t_pool.tile([128, 512], F32, tag="tp")
            for j in range(g1 - g0):
                nc.tensor.transpose(
                    tp[:, j * 128:(j + 1) * 128],
                    stage[:, (g0 + j) * 128:(g0 + j + 1) * 128],
                    ident[:])
            nc.vector.tensor_copy(
                dst[:, g0 * 128:g1 * 128].bitcast(F32R),
                tp[:, 0:(g1 - g0) * 128])

        def ensure_v(sec, tile_idx):
            ntt = sec_tiles[sec][7]
            g0 = (tile_idx // 4) * 4
            g1 = min(g0 + 4, ntt)
            key = (sec, g0)
            if key in done_v:
                return
            done_v.add(key)
            _, _, _, _, vstages, vpa_t, vpb_t, _ = sec_tiles[sec]
            # Non-first groups carry a small logical timestamp so the
            # scheduler cannot hoist them ahead of the startup-critical
            # transpose copies in the DVE FIFO (head-of-line blocking).
            late = sec == 0 and g0 > 0
            with tc.tile_wait_until(gate_v, enable=late and gate_v > 0):
                for vstage, vpt in ((vstages[0], vpa_t), (vstages[1], vpb_t)):
                    nc.vector.tensor_copy(
                        vpt[:, g0 * 65:g1 * 65].bitcast(F32R),
                        vstage[:, g0 * 65:g1 * 65])

        # ---------- compute: dr1 first (its ACT stream covers all later prep),
        # then dr2, dr4, dr8 ----------
        for di in (0, 1, 2, 3):
            dr = DRS[di]
            ls = LSS[di]
            nt = ls // 128
            off = OFFS[di]
            sec = 0 if dr == 1 else 1
            (qt, kt, _, _, _, vpa_t, vpb_t, _) = sec_tiles[sec]
            vpa = vpa_t[:].rearrange("p (t e) -> p t e", e=65)
            vpb = vpb_t[:].rearrange("p (t e) -> p t e", e=65)
            toff = TILE_OFF[dr]
            b0 = toff * 128

            cw = min(512, ls)
            n_chunks = ls // cw
            for ci in range(n_chunks):
                if dr == 1 and ci == 1:
                    with tc.tile_wait_until(gate_small, enable=gate_small > 0):
                        load_section(1, SOFF, NT_S, [(8, 14), (0, 8)])
                c0 = b0 + ci * cw
                hp = dr == 1 and ci == 0
                for u in range(toff + ci * (cw // 128),
                               toff + (ci + 1) * (cw // 128)):
                    if hp:
                        with tc.high_priority():
                            ensure_t(sec, 'q', u)
                    else:
                        ensure_t(sec, 'q', u)
                oa = o_pool.tile([65, cw], F32, tag="oa")
                ob = o_pool.tile([65, cw], F32, tag="ob")
                for kti in range(nt):
                    kc = b0 + kti * 128
                    if hp and kti == 0:
                        with tc.high_priority():
                            ensure_t(sec, 'k', toff + kti)
                            ensure_v(sec, toff + kti)
                    else:
                        ensure_t(sec, 'k', toff + kti)
                        ensure_v(sec, toff + kti)
                    s = s_pool.tile([128, 1024], F32, tag="s")
                    nc.tensor.matmul(
                        s[:, 0:cw],
                        kt[0:64, kc:kc + 128].bitcast(F32R),
                        qt[0:64, c0:c0 + cw].bitcast(F32R),
                        start=True, stop=True, tile_position=(0, 0))
                    nc.tensor.matmul(
                        s[:, 512:512 + cw],
                        kt[64:128, kc:kc + 128].bitcast(F32R),
                        qt[64:128, c0:c0 + cw].bitcast(F32R),
                        start=True, stop=True, tile_position=(64, 0))
                    p = p_pool.tile([128, 1024], F32, tag="p")
                    s_in = s[:].rearrange("x (two c) -> x two c", two=2)[:, :, 0:cw]
                    p_out = p[:, 0:2 * cw].rearrange("x (two c) -> x two c", two=2)
                    nc.scalar.activation(p_out.bitcast(F32R), s_in, EXP)
                    first, last = kti == 0, kti == nt - 1
                    nc.tensor.matmul(
                        oa[:], vpa[:, toff + kti, :].bitcast(F32R),
                        p[:, 0:cw].bitcast(F32R), start=first, stop=last)
                    nc.tensor.matmul(
                        ob[:], vpb[:, toff + kti, :].bitcast(F32R),
                        p[:, cw:2 * cw].bitcast(F32R), start=first, stop=last)
                    if kti == 0:
                        flush_pending()
                pending_out.append(
                    lambda pa=pa, pb=pb, oa=oa, ob=ob, off=off,
                           c0=ci * cw, cw=cw:
                    emit_output(pa, pb, oa, ob, off, c0, cw))

    flush_pending()
    for p in reversed(ctx_pools):
        p.__exit__(None, None, None)


_NC_CACHE = None


def _build_module(repeat=None):
    global _NC_CACHE
    import os
    if repeat is None:
        repeat = int(os.environ.get("KREPEAT", "1"))
    if _NC_CACHE is not None:
        return _NC_CACHE
    nc = bacc.Bacc("TRN2", target_bir_lowering=False, debug=False)
    q_ap = nc.dram_tensor("q", [PAIRS, TOT, D], F32, kind="ExternalInput").ap()
    k_ap = nc.dram_tensor("k", [PAIRS, TOT, D], F32, kind="ExternalInput").ap()
    v_ap = nc.dram_tensor("v", [PAIRS, TOT, D], F32, kind="ExternalInput").ap()
    o_ap = nc.dram_tensor("o", [PAIRS, TOT, D], F32, kind="ExternalOutput").ap()
    import os as _os
    gs = float(_os.environ.get("KGATES", "0.030"))
    gv = float(_os.environ.get("KGATEV", "0.008"))
    with tile.TileContext(nc) as tc:
        for _ in range(repeat):
            _build_kernel_body(tc, q_ap, k_ap, v_ap, o_ap, gate_small=gs, gate_v=gv)
        if repeat == 0:
            # minimal no-op body touching the tensors so compilation succeeds
            with tc.tile_pool(name="nul", bufs=1) as np_:
                t = np_.tile([1, 64], F32)
                nc.sync.dma_start(t[:], q_ap[0, 0:1, :])
                nc.sync.dma_start(o_ap[0, 0:1, :], t[:])
    nc.compile()
    _NC_CACHE = nc
    return nc


def _pack_inputs(query, key, value):
    """Returns in_maps: list of N_CORES dicts with q/k/v [PAIRS, TOT, D]."""
    in_maps = []
    for c in range(N_CORES):
        qm = np.empty((PAIRS, TOT, D), np.float32)
        km = np.empty((PAIRS, TOT, D), np.float32)
        vm = np.empty((PAIRS, TOT, D), np.float32)
        for i in range(PAIRS):
            bh = 4 * c + i
            h = bh % H
            for di, dr in enumerate(DRS):
                r = h // (H // dr)
                sl = slice(OFFS[di], OFFS[di] + LSS[di])
                qm[i, sl] = query[bh, r::dr]
                km[i, sl] = key[bh, r::dr]
                vm[i, sl] = value[bh, r::dr]
        in_maps.append({"q": qm, "k": km, "v": vm})
    return in_maps


def _unpack_outputs(results):
    """results: list of N_CORES dicts with 'o' [PAIRS, TOT, D] ->
    full output [B, L, H*D]."""
    out = np.zeros((B, L, H, D), np.float32)
    for c in range(N_CORES):
        o = results[c]["o"]
        for i in range(PAIRS):
            bh = 4 * c + i
            b, h = bh // H, bh % H
            for di, dr in enumerate(DRS):
                r = h // (H // dr)
                sl = slice(OFFS[di], OFFS[di] + LSS[di])
                out[b, r::dr, h] += o[i, sl]
    return out.reshape(B, L, H * D)


def kernel(query, key, value):
    query = np.asarray(query, dtype=np.float32)
    key = np.asarray(key, dtype=np.float32)
    value = np.asarray(value, dtype=np.float32)
    nc = _build_module()
    in_maps = _pack_inputs(query, key, value)
    res = run_bass_kernel_spmd(nc, in_maps, core_ids=list(range(N_CORES)))
    return _unpack_outputs(res.results)
